# revision 12
# baseline (speedup 1.0000x reference)
"""Trainium2 Bass kernel for nn_Attention_5583457485032.

Computes, for each of 2 heads (W[i] is (256,256)), iterated twice:
    temp = mean(xi, 0);  h = tanh(temp @ Wi);  s = xi @ h.T
    att = sigmoid(s / max(|s|, 1e-12))   # == sigmoid(sign(s))
    out = att.T @ xi;  xi = xi * att
and returns concat of head outputs, shape (1, 512).

Key restructuring (algebraically exact):
  - round-2 mean(xi) == out1 / N, so xi never needs materializing
  - att == sigmoid(sign(s)) since s/max(|s|,eps) == sign(s) for |s|>eps
  - per head: out2 = sum_i att1_i att2_i x_i with
      s1_i = x_i . h1, att1 = sigm(sign(s1)), out1 = sum att1_i x_i
      s2 sign == sign(x_i . h2) (att1 > 0)
Distribution: shard x row-wise over 8 cores; colsum and the (2,256)
out1 partials go through AllReduce; final out2 partials are summed on
host. Both heads are batched into the same matmuls.

3 passes over x per core (v0: f32 streaming from HBM each pass):
  A: colsum (ones-matmul)      -> AR -> h1 = tanh(colsum/N @ W)
  B: per 128-row tile: PE-transpose x, scores = xT.T @ h_col,
     att1 = sigmoid(sign(s)), outacc += att1.T @ x   -> AR -> h2
  C: same with h2; weights att1*att2.
"""

import os
import numpy as np

N_CORES = 8
N_TOTAL = 200000
D = 256
H = 2
P = 128
TPC = 8  # tiles per DMA chunk


def build_kernel(n_rows, n_cores, n_total=None):
    import concourse.bass as bass
    import concourse.mybir as mybir

    F32 = mybir.dt.float32
    AF = mybir.ActivationFunctionType
    ALU = mybir.AluOpType

    if n_total is None:
        n_total = n_rows * n_cores

    T = (n_rows + P - 1) // P
    n_chunks = (T + TPC - 1) // TPC
    chunk_tiles = [list(range(c * TPC, min(T, (c + 1) * TPC))) for c in range(n_chunks)]

    def rows_of(t):
        return min(P, n_rows - t * P)

    nc = bass.Bass()
    x_ext = nc.declare_dram_parameter("x", [n_rows, D], F32, isOutput=False)
    w_ext = nc.declare_dram_parameter("W", [H, D, D], F32, isOutput=False)
    out_ext = nc.declare_dram_parameter("out", [H, D], F32, isOutput=True)

    cs_dram = nc.dram_tensor("cs_dram", [1, D], F32)
    cs_ar = nc.dram_tensor("cs_ar", [1, D], F32)
    o1_dram = nc.dram_tensor("o1_dram", [H, D], F32)
    o1_ar = nc.dram_tensor("o1_ar", [H, D], F32)

    sb = nc.alloc_sbuf_tensor
    xbuf = [sb(f"xbuf{b}", [P, TPC * D], F32) for b in range(2)]
    xts = [sb(f"xts{b}", [P, 2 * P], F32) for b in range(2)]
    ones = sb("ones", [P, P], F32)
    iden = sb("iden", [P, P], F32)
    wsb = sb("wsb", [P, H * 2 * 2 * P], F32)  # chunk (h,dc,oc) at col ((h*2+dc)*2+oc)*128
    cs_row = sb("cs_row", [1, D], F32)
    o1_sb = sb("o1_sb", [H, D], F32)
    o1r_sb = sb("o1r_sb", [H, D], F32)
    cs_col = sb("cs_col", [P, 2], F32)        # [:, dc]
    cs2_col = sb("cs2_col", [P, 2 * H], F32)  # [:, dc*H + h]
    hcol = sb("hcol", [P, 2 * H], F32)        # [:, oc*H + h]
    sgn = [sb(f"sgn{b}", [P, H], F32) for b in range(2)]
    att1 = sb("att1", [P, T * H], F32)        # tile t at cols [t*H, t*H+H)
    att2 = [sb(f"att2{b}", [P, H], F32) for b in range(2)]
    wv = [sb(f"wv{b}", [P, H], F32) for b in range(2)]
    out2_sb = sb("out2_sb", [H, D], F32)

    ps = nc.alloc_psum_tensor
    xtp = [ps(f"xtp{b}", [P, 2 * P], F32) for b in range(2)]
    spsum = [ps(f"spsum{b}", [P, H], F32) for b in range(2)]
    outacc = ps("outacc", [H, D], F32)
    cspsum = ps("cspsum", [1, D], F32)
    cst = ps("cst", [P, 2 * H], F32)
    ht = ps("ht", [P, 2 * H], F32)

    sems = {k: nc.alloc_semaphore(k) for k in
            ("dma_w", "dma_x0", "dma_x1", "dma_m", "pe", "act", "dve", "cc")}

    ENGS = ("sp", "pe", "act", "dve", "pool")

    class Sched:
        def __init__(self, plan=None):
            self.plan = plan
            self.ctr = {k: 0 for k in sems}
            self.ev = {} if plan is None else plan
            self.ops = {e: [] for e in ENGS}
            self.seen = {e: {} for e in ENGS}

        def inst(self, eng, sem, thunk, key=None, step=1):
            self.ctr[sem] += step
            v = self.ctr[sem]
            if self.plan is None:
                if key is not None:
                    assert key not in self.ev, key
                    self.ev[key] = (sem, v)
            else:
                if key is not None:
                    assert self.ev[key] == (sem, v), (key, self.ev[key], sem, v)
                self.ops[eng].append(("i", thunk, sem, step))
            return v

        def wait(self, eng, key):
            if self.plan is None:
                return
            sem, v = self.ev[key]
            if v <= 0 or self.seen[eng].get(sem, 0) >= v:
                return
            self.seen[eng][sem] = v
            self.ops[eng].append(("w", sem, v))

    def chunk_load(S, c, key):
        r0 = c * TPC * P
        r1 = min(n_rows, (c + 1) * TPC * P)
        rows = r1 - r0
        nt = rows // P
        tail = rows - nt * P
        b = c % 2
        sem = f"dma_x{b}"
        if nt:
            def f(b=b, r0=r0, nt=nt):
                src = x_ext[r0:r0 + nt * P, :].rearrange("(n p) m -> p n m", p=P)
                dst = xbuf[b][:, 0:nt * D]
                return nc.sync.dma_start(out=dst, in_=src)
            S.inst("sp", sem, f, step=16, key=None if tail else key)
        if tail:
            def f2(b=b, r0=r0, nt=nt, tail=tail):
                return nc.sync.dma_start(
                    out=xbuf[b][0:tail, nt * D:(nt + 1) * D],
                    in_=x_ext[r0 + nt * P:r0 + nt * P + tail, :])
            S.inst("sp", sem, f2, step=16, key=key)

    def h_phase(S, tag, cc_key, col_sb, col_idx):
        """Transposes of the all-reduced (rows,256) vector into column
        layout, then 8 W-matmuls + tanh into hcol. tag in ('h1','h2')."""
        if tag == "h1":
            row_sb, nr = cs_row, 1
        else:
            row_sb, nr = o1r_sb, H
        # dma the AR result back
        S.wait("sp", cc_key)
        src_ar = cs_ar if tag == "h1" else o1_ar
        S.inst("sp", "dma_m",
               lambda row_sb=row_sb, nr=nr, src_ar=src_ar:
               nc.sync.dma_start(out=row_sb[0:nr, :], in_=src_ar[:, :]),
               step=16, key=("dma", tag + "_in"))
        # PE transposes into cst
        S.wait("pe", ("dma", tag + "_in"))
        S.wait("pe", ("dve", "const"))
        for dc in range(2):
            S.inst("pe", "pe",
                   lambda dc=dc, row_sb=row_sb, nr=nr:
                   nc.tensor.transpose(
                       cst[:, dc * nr:(dc + 1) * nr],
                       row_sb[0:nr, dc * P:(dc + 1) * P],
                       iden[0:nr, 0:nr]),
                   key=("pe", tag + "_tr") if dc == 1 else None)
        # DVE copy cst -> col_sb
        S.wait("dve", ("pe", tag + "_tr"))
        S.inst("dve", "dve",
               lambda col_sb=col_sb, nr=nr:
               nc.vector.tensor_copy(col_sb[:, 0:2 * nr], cst[:, 0:2 * nr]),
               key=("dve", tag + "_col"))
        # PE h-matmuls
        S.wait("pe", ("dma", "W"))
        S.wait("pe", ("dve", tag + "_col"))
        for h in range(H):
            for oc in range(2):
                for dc in range(2):
                    widx = (h * 2 + dc) * 2 + oc
                    S.inst("pe", "pe",
                           lambda h=h, oc=oc, dc=dc, widx=widx, col_sb=col_sb:
                           nc.tensor.matmul(
                               ht[:, oc * H + h:oc * H + h + 1],
                               wsb[:, widx * P:(widx + 1) * P],
                               col_sb[:, col_idx(dc, h):col_idx(dc, h) + 1],
                               start=(dc == 0), stop=(dc == 1),
                               skip_group_check=True),
                           key=("pe", tag + "_mm")
                           if (h, oc, dc) == (H - 1, 1, 1) else None)
        # ACT tanh (scale = 1/N_total applied before tanh)
        S.wait("act", ("pe", tag + "_mm"))
        S.inst("act", "act",
               lambda: nc.scalar.activation(
                   hcol[:, :], ht[:, :], AF.Tanh, scale=1.0 / float(n_total)),
               key=("act", tag))

    def pass_bc(S, tag):
        """Main per-tile pipeline for pass B (tag='B') or C (tag='C')."""
        is_c = tag == "C"
        load = ("dma", tag + "load")
        htag = "h1" if tag == "B" else "h2"

        def tr_tile(t):
            c = t // TPC
            b = t % 2
            r = rows_of(t)
            tic = t - c * TPC
            S.wait("pe", (load[0], load[1], c))
            if t >= 2:
                S.wait("pe", ("dve", tag + "_copy", t - 2))
            for oc in range(2):
                S.inst("pe", "pe",
                       lambda t=t, b=b, r=r, tic=tic, oc=oc:
                       nc.tensor.transpose(
                           xtp[b][:, oc * P:oc * P + r],
                           xbuf[(t // TPC) % 2][0:r, tic * D + oc * P:tic * D + (oc + 1) * P],
                           iden[0:r, 0:r]),
                       key=("pe", tag + "_tr", t) if oc == 1 else None)

        def mms_tile(t):
            b = t % 2
            r = rows_of(t)
            S.wait("pe", ("dve", tag + "_copy", t))
            if t == 0:
                S.wait("pe", ("act", htag))
            if t >= 2:
                S.wait("pe", ("act", tag + "_sig", t - 2))
            for oc in range(2):
                S.inst("pe", "pe",
                       lambda t=t, b=b, r=r, oc=oc:
                       nc.tensor.matmul(
                           spsum[b][0:r, :],
                           xts[b][:, oc * P:oc * P + r],
                           hcol[:, oc * H:(oc + 1) * H],
                           start=(oc == 0), stop=(oc == 1),
                           skip_group_check=True),
                       key=("pe", tag + "_mmS", t) if oc == 1 else None)

        def mmout_tile(t):
            b = t % 2
            r = rows_of(t)
            c = t // TPC
            if is_c:
                S.wait("pe", ("dve", "C_w", t))
                lhs = lambda t=t, b=b, r=r: wv[b][0:r, :]
            else:
                S.wait("pe", ("act", "B_sig", t))
                lhs = lambda t=t, r=r: att1[0:r, t * H:(t + 1) * H]
            tic = t - c * TPC
            S.inst("pe", "pe",
                   lambda t=t, r=r, c=c, tic=tic, lhs=lhs:
                   nc.tensor.matmul(
                       outacc[:, :],
                       lhs(),
                       xbuf[c % 2][0:r, tic * D:(tic + 1) * D],
                       start=(t == 0), stop=(t == T - 1),
                       skip_group_check=True),
                   key=("pe", tag + "_mmOut", t))

        def act_tile(t):
            b = t % 2
            r = rows_of(t)
            S.wait("act", ("pe", tag + "_mmS", t))
            if is_c and t >= 2:
                S.wait("act", ("dve", "C_w", t - 2))
            S.inst("act", "act",
                   lambda t=t, b=b, r=r:
                   nc.scalar.activation(sgn[b][0:r, :], spsum[b][0:r, :], AF.Sign),
                   key=("act", tag + "_sgn", t))
            S.wait("act", ("act", tag + "_sgn", t))
            if is_c:
                dst = lambda t=t, b=b, r=r: att2[b][0:r, :]
            else:
                dst = lambda t=t, r=r: att1[0:r, t * H:(t + 1) * H]
            S.inst("act", "act",
                   lambda t=t, b=b, r=r, dst=dst:
                   nc.scalar.activation(dst(), sgn[b][0:r, :], AF.Sigmoid),
                   key=("act", tag + "_sig", t))

        def dve_copy(t):
            b = t % 2
            S.wait("dve", ("pe", tag + "_tr", t))
            S.inst("dve", "dve",
                   lambda t=t, b=b:
                   nc.vector.tensor_copy(xts[b][:, :], xtp[b][:, :]),
                   key=("dve", tag + "_copy", t))

        def dve_w(t):
            b = t % 2
            r = rows_of(t)
            S.wait("dve", ("act", "C_sig", t))
            S.inst("dve", "dve",
                   lambda t=t, b=b, r=r:
                   nc.vector.tensor_mul(
                       wv[b][0:r, :], att2[b][0:r, :],
                       att1[0:r, t * H:(t + 1) * H]),
                   key=("dve", "C_w", t))

        # PE stream (software-pipelined)
        tr_tile(0)
        if T > 1:
            tr_tile(1)
        for t in range(T):
            mms_tile(t)
            if t + 2 < T:
                tr_tile(t + 2)
            if t >= 1:
                mmout_tile(t - 1)
        mmout_tile(T - 1)

        # ACT stream
        for t in range(T):
            act_tile(t)

        # DVE stream
        dve_copy(0)
        for t in range(T):
            if t + 1 < T:
                dve_copy(t + 1)
            if is_c:
                dve_w(t)

    def col_idx_h1(dc, h):
        return dc

    def col_idx_h2(dc, h):
        return dc * H + h

    def sched(S):
        # ---- preamble (constants on gpsimd, W loads) ----
        S.inst("pool", "dve", lambda: nc.gpsimd.memset(ones.ap(), 1.0),
               key=("dve", "ones"))
        S.wait("pool", ("dve", "ones"))
        S.inst("pool", "dve",
               lambda: nc.gpsimd.affine_select(
                   iden.ap(), ones.ap(), pattern=[[-1, P]],
                   compare_op=ALU.is_equal, fill=0.0, base=0,
                   channel_multiplier=1),
               key=("dve", "const"))
        for h in range(H):
            for dc in range(2):
                for oc in range(2):
                    widx = (h * 2 + dc) * 2 + oc
                    S.inst("sp", "dma_w",
                           lambda h=h, dc=dc, oc=oc, widx=widx:
                           nc.sync.dma_start(
                               out=wsb[:, widx * P:(widx + 1) * P],
                               in_=w_ext[h, dc * P:(dc + 1) * P, oc * P:(oc + 1) * P]),
                           step=16,
                           key=("dma", "W") if widx == H * 4 - 1 else None)

        # ---- phase A: loads ----
        for c in range(n_chunks):
            if c >= 2:
                S.wait("sp", ("pe", "A_chunk", c - 2))
            chunk_load(S, c, ("dma", "Aload", c))
        # ---- phase A: colsum ----
        for c in range(n_chunks):
            S.wait("pe", ("dma", "Aload", c))
            if c == 0:
                S.wait("pe", ("dve", "const"))
            for t in chunk_tiles[c]:
                r = rows_of(t)
                tic = t - c * TPC
                S.inst("pe", "pe",
                       lambda c=c, r=r, tic=tic, t=t:
                       nc.tensor.matmul(
                           cspsum[0:1, :], ones[0:r, 0:1],
                           xbuf[c % 2][0:r, tic * D:(tic + 1) * D],
                           start=(t == 0), stop=(t == T - 1),
                           skip_group_check=True),
                       key=("pe", "A_chunk", c) if t == chunk_tiles[c][-1] else None)
        # colsum -> AR
        S.wait("act", ("pe", "A_chunk", n_chunks - 1))
        S.inst("act", "act",
               lambda: nc.scalar.copy(cs_row[0:1, :], cspsum[0:1, :]),
               key=("act", "A_copy"))
        S.wait("sp", ("act", "A_copy"))
        S.inst("sp", "dma_m",
               lambda: nc.sync.dma_start(out=cs_dram[:, :], in_=cs_row[0:1, :]),
               step=16, key=("dma", "cs_out"))
        S.wait("pool", ("dma", "cs_out"))
        S.inst("pool", "cc",
               lambda: nc.gpsimd.collective_compute(
                   "AllReduce", mybir.AluOpType.add,
                   replica_groups=[list(range(n_cores))],
                   ins=[cs_dram[:, :]], outs=[cs_ar[:, :]]),
               key=("cc", "ar1"))
        # prefetch B chunks 0,1 while AR in flight
        S.wait("sp", ("pe", "A_chunk", n_chunks - 1))
        chunk_load(S, 0, ("dma", "Bload", 0))
        if n_chunks > 1:
            chunk_load(S, 1, ("dma", "Bload", 1))
        # h1
        h_phase(S, "h1", ("cc", "ar1"), cs_col, col_idx_h1)
        # ---- phase B ----
        for c in range(2, n_chunks):
            S.wait("sp", ("pe", "B_mmOut", chunk_tiles[c - 2][-1]))
            chunk_load(S, c, ("dma", "Bload", c))
        pass_bc(S, "B")
        # out1 -> AR
        S.wait("act", ("pe", "B_mmOut", T - 1))
        S.inst("act", "act",
               lambda: nc.scalar.copy(o1_sb[0:H, :], outacc[:, :]),
               key=("act", "o1_copy"))
        S.wait("sp", ("act", "o1_copy"))
        S.inst("sp", "dma_m",
               lambda: nc.sync.dma_start(out=o1_dram[:, :], in_=o1_sb[0:H, :]),
               step=16, key=("dma", "o1_out"))
        S.wait("pool", ("dma", "o1_out"))
        S.inst("pool", "cc",
               lambda: nc.gpsimd.collective_compute(
                   "AllReduce", mybir.AluOpType.add,
                   replica_groups=[list(range(n_cores))],
                   ins=[o1_dram[:, :]], outs=[o1_ar[:, :]]),
               key=("cc", "ar2"))
        # prefetch C chunks 0,1
        S.wait("sp", ("pe", "B_mmOut", T - 1))
        chunk_load(S, 0, ("dma", "Cload", 0))
        if n_chunks > 1:
            chunk_load(S, 1, ("dma", "Cload", 1))
        # h2
        h_phase(S, "h2", ("cc", "ar2"), cs2_col, col_idx_h2)
        # ---- phase C ----
        for c in range(2, n_chunks):
            S.wait("sp", ("pe", "C_mmOut", chunk_tiles[c - 2][-1]))
            chunk_load(S, c, ("dma", "Cload", c))
        pass_bc(S, "C")
        # final out
        S.wait("act", ("pe", "C_mmOut", T - 1))
        S.inst("act", "act",
               lambda: nc.scalar.copy(out2_sb[0:H, :], outacc[:, :]),
               key=("act", "out_copy"))
        S.wait("sp", ("act", "out_copy"))
        S.inst("sp", "dma_m",
               lambda: nc.sync.dma_start(out=out_ext[:, :], in_=out2_sb[0:H, :]),
               step=16, key=("dma", "out_final"))
        S.wait("sp", ("dma", "out_final"))

    plan = Sched()
    sched(plan)
    emit = Sched(plan.ev)
    sched(emit)

    eng_map = {
        "sp": nc.sync, "pe": nc.tensor, "act": nc.scalar,
        "dve": nc.vector, "pool": nc.gpsimd,
    }

    def run_ops(eng_name):
        eng = eng_map[eng_name]
        def body(_engine):
            for op in emit.ops[eng_name]:
                if op[0] == "w":
                    _, sem, v = op
                    eng.wait_ge(sems[sem], v)
                else:
                    _, thunk, sem, step = op
                    bi = thunk()
                    bi.then_inc(sems[sem], step)
        return body

    with nc.Block() as block:
        block.sync(run_ops("sp"))
        block.gpsimd(run_ops("pool"))
        block.tensor(run_ops("pe"))
        block.scalar(run_ops("act"))
        block.vector(run_ops("dve"))

    return nc


_NC_CACHE = {}


def _get_nc(n_rows, n_cores, n_total):
    key = (n_rows, n_cores, n_total)
    if key not in _NC_CACHE:
        _NC_CACHE[key] = build_kernel(n_rows, n_cores, n_total)
    return _NC_CACHE[key]


def kernel(x, W):
    from concourse.bass_utils import run_bass_kernel_spmd

    x = np.ascontiguousarray(np.asarray(x, dtype=np.float32))
    W = np.ascontiguousarray(np.asarray(W, dtype=np.float32))
    n, d = x.shape
    assert n % N_CORES == 0 and d == D
    n_rows = n // N_CORES

    nc = _get_nc(n_rows, N_CORES, n)
    in_maps = [
        {"x": x[i * n_rows:(i + 1) * n_rows], "W": W} for i in range(N_CORES)
    ]
    res = run_bass_kernel_spmd(nc, in_maps, core_ids=list(range(N_CORES)))
    total = np.zeros((H, D), dtype=np.float64)
    for i in range(N_CORES):
        total += res.results[i]["out"].astype(np.float64)
    return total.astype(np.float32).reshape(1, H * D)


if __name__ == "__main__":
    rng = np.random.default_rng(0)
    x = rng.standard_normal((N_TOTAL, D)).astype(np.float32)
    W = (rng.standard_normal((H, D, D)) * np.sqrt(2.0 / (D + D))).astype(np.float32)
    out = kernel(x=x, W=W)
    print(out.shape, out[0, :4])


# revision 15
# speedup vs baseline: 3.8100x; 3.8100x over previous
"""Trainium2 Bass kernel for nn_Attention_5583457485032.

Computes, for each of 2 heads (W[i] is (256,256)), iterated twice:
    temp = mean(xi, 0);  h = tanh(temp @ Wi);  s = xi @ h.T
    att = sigmoid(s / max(|s|, 1e-12))   # == sigmoid(sign(s))
    out = att.T @ xi;  xi = xi * att
and returns concat of head outputs, shape (1, 512).

Key restructuring (algebraically exact):
  - round-2 mean(xi) == out1 / N, so xi never needs materializing
  - att == sigmoid(sign(s)) since s/max(|s|,eps) == sign(s) for |s|>eps
  - per head: out2 = sum_i att1_i att2_i x_i with
      s1_i = x_i . h1, att1 = sigm(sign(s1)), out1 = sum att1_i x_i
      s2 sign == sign(x_i . h2) (att1 > 0)
Distribution: shard x row-wise over 8 cores; colsum and the (2,256)
out1 partials go through AllReduce; final out2 partials are summed on
host. Both heads are batched into the same matmuls.

v1: single HBM read of x. DMA casts f32->bf16 (SWDGE) into a resident
SBUF copy x_nat; pass A also builds a PE-transposed resident copy xts
(per 128-row tile) while computing the colsum. Passes B and C then run
entirely from SBUF: scores via xts-stationary matmuls, per-8-tile
batched sign/sigmoid on ScalarE, weighted column-sum accumulation into
PSUM. All big matmuls are bf16 (single-pass on the PE, double-pumped).
"""

import os
import numpy as np

N_CORES = 8
N_TOTAL = 200000
D = 256
H = 2
P = 128
TPC = 8   # tiles per DMA chunk
G = 8     # tiles per sign/sigmoid group


def build_kernel(n_rows, n_cores, n_total=None):
    import concourse.bass as bass
    import concourse.mybir as mybir

    F32 = mybir.dt.float32
    BF16 = mybir.dt.bfloat16
    AF = mybir.ActivationFunctionType
    ALU = mybir.AluOpType

    if n_total is None:
        n_total = n_rows * n_cores

    T = (n_rows + P - 1) // P
    n_chunks = (T + TPC - 1) // TPC
    chunk_tiles = [list(range(c * TPC, min(T, (c + 1) * TPC))) for c in range(n_chunks)]
    n_groups = (T + G - 1) // G
    group_tiles = [list(range(g * G, min(T, (g + 1) * G))) for g in range(n_groups)]

    def rows_of(t):
        return min(P, n_rows - t * P)

    nc = bass.Bass()
    x_ext = nc.declare_dram_parameter("x", [n_rows, D], F32, isOutput=False)
    w_ext = nc.declare_dram_parameter("W", [H, D, D], F32, isOutput=False)
    out_ext = nc.declare_dram_parameter("out", [H, D], F32, isOutput=True)

    cs_dram = nc.dram_tensor("cs_dram", [1, D], F32)
    cs_ar = nc.dram_tensor("cs_ar", [1, D], F32)
    o1_dram = nc.dram_tensor("o1_dram", [H, D], F32)
    o1_ar = nc.dram_tensor("o1_ar", [H, D], F32)

    sb = nc.alloc_sbuf_tensor
    x_nat = sb("x_nat", [P, T * D], BF16)      # tile t at cols [t*D,(t+1)*D)
    xts = sb("xts", [P, T * D], BF16)          # tile t: [t*D + oc*P + row]
    ones_f = sb("ones_f", [P, P], F32)
    iden_f = sb("iden_f", [P, P], F32)
    ones_b = sb("ones_b", [P, P], BF16)
    iden_b = sb("iden_b", [P, P], BF16)
    wsb = sb("wsb", [P, H * 2 * 2 * P], F32)   # (h,dc,oc) at col ((h*2+dc)*2+oc)*128
    cs_row = sb("cs_row", [1, D], F32)
    o1_sb = sb("o1_sb", [H, D], F32)           # also reused for final out2 staging
    o1r_sb = sb("o1r_sb", [H, D], F32)
    cs_col = sb("cs_col", [P, 2], F32)         # [:, dc]
    cs2_col = sb("cs2_col", [P, 2 * H], F32)   # [:, dc*H + h]
    hcol = sb("hcol", [P, 2 * H], BF16)        # [:, oc*H + h]
    sgn = [sb(f"sgn{b}", [P, G * H], F32) for b in range(2)]
    att1 = sb("att1", [P, T * H], BF16)        # tile t at cols [t*H, t*H+H)
    att2 = [sb(f"att2{b}", [P, G * H], BF16) for b in range(2)]
    wv = [sb(f"wv{b}", [P, G * H], BF16) for b in range(2)]

    ps = nc.alloc_psum_tensor
    xtp = [ps(f"xtp{b}", [P, 2 * P], BF16) for b in range(2)]
    spsum = [ps(f"spsum{b}", [P, G * H], F32) for b in range(2)]
    outacc = ps("outacc", [H, D], F32)
    cspsum = ps("cspsum", [1, D], F32)
    cst = ps("cst", [P, 2 * H], F32)
    ht = ps("ht", [P, 2 * H], F32)

    sems = {k: nc.alloc_semaphore(k) for k in
            ("dma_w", "dma_x0", "dma_x1", "dma_m", "pe", "act", "dve", "cc")}

    ENGS = ("sp", "pe", "act", "dve", "pool")

    class Sched:
        def __init__(self, plan=None):
            self.plan = plan
            self.ctr = {k: 0 for k in sems}
            self.ev = {} if plan is None else plan
            self.ops = {e: [] for e in ENGS}
            self.seen = {e: {} for e in ENGS}

        def inst(self, eng, sem, thunk, key=None, step=1):
            self.ctr[sem] += step
            v = self.ctr[sem]
            if self.plan is None:
                if key is not None:
                    assert key not in self.ev, key
                    self.ev[key] = (sem, v)
            else:
                if key is not None:
                    assert self.ev[key] == (sem, v), (key, self.ev[key], sem, v)
                self.ops[eng].append(("i", thunk, sem, step))
            return v

        def wait(self, eng, key):
            if self.plan is None:
                return
            sem, v = self.ev[key]
            if v <= 0 or self.seen[eng].get(sem, 0) >= v:
                return
            self.seen[eng][sem] = v
            self.ops[eng].append(("w", sem, v))

    def chunk_load(S, c):
        """SWDGE (gpsimd) DMA with f32->bf16 cast straight into x_nat."""
        r0 = c * TPC * P
        r1 = min(n_rows, (c + 1) * TPC * P)
        rows = r1 - r0
        nt = rows // P
        tail = rows - nt * P
        sem = f"dma_x{c % 2}"
        key = ("dma", "load", c)
        if nt:
            def f(c=c, r0=r0, nt=nt):
                src = x_ext[r0:r0 + nt * P, :].rearrange("(n p) m -> p n m", p=P)
                dst = x_nat[:, c * TPC * D:c * TPC * D + nt * D]
                return nc.gpsimd.dma_start(out=dst, in_=src)
            S.inst("pool", sem, f, step=16, key=None if tail else key)
        if tail:
            def f2(c=c, r0=r0, nt=nt, tail=tail):
                base = (c * TPC + nt) * D
                return nc.gpsimd.dma_start(
                    out=x_nat[0:tail, base:base + D],
                    in_=x_ext[r0 + nt * P:r0 + nt * P + tail, :])
            S.inst("pool", sem, f2, step=16, key=key)

    def h_phase(S, tag, cc_key, col_sb, col_idx):
        """AR result -> column layout -> 8 W-matmuls (f32) + tanh -> hcol."""
        if tag == "h1":
            row_sb, nr = cs_row, 1
        else:
            row_sb, nr = o1r_sb, H
        S.wait("sp", cc_key)
        src_ar = cs_ar if tag == "h1" else o1_ar
        S.inst("sp", "dma_m",
               lambda row_sb=row_sb, nr=nr, src_ar=src_ar:
               nc.sync.dma_start(out=row_sb[0:nr, :], in_=src_ar[:, :]),
               step=16, key=("dma", tag + "_in"))
        S.wait("pe", ("dma", tag + "_in"))
        S.wait("pe", ("dve", "const"))
        for dc in range(2):
            S.inst("pe", "pe",
                   lambda dc=dc, row_sb=row_sb, nr=nr:
                   nc.tensor.transpose(
                       cst[:, dc * nr:(dc + 1) * nr],
                       row_sb[0:nr, dc * P:(dc + 1) * P],
                       iden_f[0:nr, 0:nr]),
                   key=("pe", tag + "_tr") if dc == 1 else None)
        S.wait("dve", ("pe", tag + "_tr"))
        S.inst("dve", "dve",
               lambda col_sb=col_sb, nr=nr:
               nc.vector.tensor_copy(col_sb[:, 0:2 * nr], cst[:, 0:2 * nr]),
               key=("dve", tag + "_col"))
        S.wait("pe", ("dma", "W"))
        S.wait("pe", ("dve", tag + "_col"))
        for h in range(H):
            for oc in range(2):
                for dc in range(2):
                    widx = (h * 2 + dc) * 2 + oc
                    S.inst("pe", "pe",
                           lambda h=h, oc=oc, dc=dc, widx=widx, col_sb=col_sb:
                           nc.tensor.matmul(
                               ht[:, oc * H + h:oc * H + h + 1],
                               wsb[:, widx * P:(widx + 1) * P],
                               col_sb[:, col_idx(dc, h):col_idx(dc, h) + 1],
                               start=(dc == 0), stop=(dc == 1),
                               skip_group_check=True),
                           key=("pe", tag + "_mm")
                           if (h, oc, dc) == (H - 1, 1, 1) else None)
        S.wait("act", ("pe", tag + "_mm"))
        S.inst("act", "act",
               lambda: nc.scalar.activation(
                   hcol[:, :], ht[:, :], AF.Tanh, scale=1.0 / float(n_total)),
               key=("act", tag))

    def pass_bc(S, tag):
        """Per-tile scores+weighted-sum from resident SBUF copies.
        ACT sign/sigmoid batched per group of G tiles."""
        is_c = tag == "C"
        htag = "h1" if tag == "B" else "h2"

        def grp(t):
            return t // G

        def mms_tile(t):
            g = grp(t)
            b = g % 2
            r = rows_of(t)
            col = (t - g * G) * H
            if t == 0:
                S.wait("pe", ("act", htag))
                if tag == "B":
                    # all resident xts copies must have landed (the ACT-side
                    # ones are covered transitively through h1; wait on the
                    # last DVE-side copy explicitly)
                    last_dve = T - 1 if (T - 1) % 2 == 0 else T - 2
                    if last_dve >= 0:
                        S.wait("pe", ("cp", "A_copy", last_dve))
            if g >= 2 and t == group_tiles[g][0]:
                S.wait("pe", ("act", tag + "_sig", g - 2))
            for oc in range(2):
                S.inst("pe", "pe",
                       lambda t=t, b=b, r=r, col=col, oc=oc:
                       nc.tensor.matmul(
                           spsum[b][0:r, col:col + H],
                           xts[:, t * D + oc * P:t * D + oc * P + r],
                           hcol[:, oc * H:(oc + 1) * H],
                           start=(oc == 0), stop=(oc == 1),
                           skip_group_check=True),
                       key=("pe", tag + "_mmS", t) if oc == 1 else None)

        def mmout_tile(t):
            g = grp(t)
            r = rows_of(t)
            if is_c:
                S.wait("pe", ("dve", "C_w", g))
                lhs = lambda t=t, r=r, g=g: wv[g % 2][0:r, (t - g * G) * H:(t - g * G) * H + H]
            else:
                S.wait("pe", ("act", "B_sig", g))
                lhs = lambda t=t, r=r: att1[0:r, t * H:(t + 1) * H]
            S.inst("pe", "pe",
                   lambda t=t, r=r, lhs=lhs:
                   nc.tensor.matmul(
                       outacc[:, :],
                       lhs(),
                       x_nat[0:r, t * D:(t + 1) * D],
                       start=(t == 0), stop=(t == T - 1),
                       skip_group_check=True),
                   key=("pe", tag + "_mmOut", t))

        def act_group(g):
            b = g % 2
            tiles = group_tiles[g]
            ncols = len(tiles) * H
            rmax = rows_of(tiles[0])
            S.wait("act", ("pe", tag + "_mmS", tiles[-1]))
            if g >= 2:
                S.wait("act", ("act", tag + "_sig", g - 2))
                if is_c:
                    S.wait("act", ("dve", "C_w", g - 2))
            S.inst("act", "act",
                   lambda g=g, b=b, rmax=rmax, ncols=ncols:
                   nc.scalar.activation(
                       sgn[b][0:rmax, 0:ncols], spsum[b][0:rmax, 0:ncols],
                       AF.Sign),
                   key=("act", tag + "_sgn", g))
            S.wait("act", ("act", tag + "_sgn", g))
            if is_c:
                dst = lambda g=g, b=b, rmax=rmax, ncols=ncols: att2[b][0:rmax, 0:ncols]
            else:
                dst = lambda g=g, rmax=rmax, ncols=ncols: \
                    att1[0:rmax, g * G * H:g * G * H + ncols]
            S.inst("act", "act",
                   lambda g=g, b=b, rmax=rmax, ncols=ncols, dst=dst:
                   nc.scalar.activation(dst(), sgn[b][0:rmax, 0:ncols],
                                        AF.Sigmoid),
                   key=("act", tag + "_sig", g))

        def dve_w(g):
            b = g % 2
            tiles = group_tiles[g]
            ncols = len(tiles) * H
            rmax = rows_of(tiles[0])
            S.wait("dve", ("act", "C_sig", g))
            S.inst("dve", "dve",
                   lambda g=g, b=b, rmax=rmax, ncols=ncols:
                   nc.vector.tensor_mul(
                       wv[b][0:rmax, 0:ncols], att2[b][0:rmax, 0:ncols],
                       att1[0:rmax, g * G * H:g * G * H + ncols]),
                   key=("dve", "C_w", g))

        # PE stream: scores for group g+1 interleave with mmOut of group g
        for t in group_tiles[0]:
            mms_tile(t)
        for g in range(n_groups):
            if g + 1 < n_groups:
                for t in group_tiles[g + 1]:
                    mms_tile(t)
            for t in group_tiles[g]:
                mmout_tile(t)
        # ACT stream
        for g in range(n_groups):
            act_group(g)
        # DVE stream
        if is_c:
            for g in range(n_groups):
                dve_w(g)

    def col_idx_h1(dc, h):
        return dc

    def col_idx_h2(dc, h):
        return dc * H + h

    def sched(S):
        # ---- preamble: zero spsum tails (DVE; partial tiles leave rows
        # uninitialized and the grouped sign/sigmoid reads full rectangles)
        S.inst("dve", "dve", lambda: nc.vector.memset(spsum[0].ap(), 0.0))
        S.inst("dve", "dve", lambda: nc.vector.memset(spsum[1].ap(), 0.0))
        # ---- preamble: constants (gpsimd), W loads (sync) ----
        S.inst("pool", "dve", lambda: nc.gpsimd.memset(ones_f.ap(), 1.0),
               key=("dve", "ones"))
        S.inst("pool", "dve", lambda: nc.gpsimd.memset(ones_b.ap(), 1.0),
               key=("dve", "ones_b"))
        S.wait("pool", ("dve", "ones"))
        S.wait("pool", ("dve", "ones_b"))
        S.inst("pool", "dve",
               lambda: nc.gpsimd.affine_select(
                   iden_f.ap(), ones_f.ap(), pattern=[[-1, P]],
                   compare_op=ALU.is_equal, fill=0.0, base=0,
                   channel_multiplier=1),
               key=("dve", "iden_f"))
        S.inst("pool", "dve",
               lambda: nc.gpsimd.affine_select(
                   iden_b.ap(), ones_b.ap(), pattern=[[-1, P]],
                   compare_op=ALU.is_equal, fill=0.0, base=0,
                   channel_multiplier=1),
               key=("dve", "const"))
        for h in range(H):
            for dc in range(2):
                for oc in range(2):
                    widx = (h * 2 + dc) * 2 + oc
                    S.inst("sp", "dma_w",
                           lambda h=h, dc=dc, oc=oc, widx=widx:
                           nc.sync.dma_start(
                               out=wsb[:, widx * P:(widx + 1) * P],
                               in_=w_ext[h, dc * P:(dc + 1) * P, oc * P:(oc + 1) * P]),
                           step=16,
                           key=("dma", "W") if widx == H * 4 - 1 else None)

        # ---- phase A: chunk loads (pool, SWDGE cast) gated 2-deep ----
        for c in range(n_chunks):
            if c >= 2:
                S.wait("pool", ("pe", "A_chunk", c - 2))
            chunk_load(S, c)
        # ---- phase A: per tile colsum + 2 transposes; copies split DVE/ACT
        for c in range(n_chunks):
            S.wait("pe", ("dma", "load", c))
            if c == 0:
                S.wait("pe", ("dve", "const"))
            for t in chunk_tiles[c]:
                r = rows_of(t)
                # colsum
                S.inst("pe", "pe",
                       lambda t=t, r=r:
                       nc.tensor.matmul(
                           cspsum[0:1, :], ones_b[0:r, 0:1],
                           x_nat[0:r, t * D:(t + 1) * D],
                           start=(t == 0), stop=(t == T - 1),
                           skip_group_check=True),
                       key=("pe", "A_cs", t))
                # transposes into xtp[t%2]
                if t >= 2:
                    S.wait("pe", ("cp", "A_copy", t - 2))
                for oc in range(2):
                    S.inst("pe", "pe",
                           lambda t=t, r=r, oc=oc:
                           nc.tensor.transpose(
                               xtp[t % 2][:, oc * P:oc * P + r],
                               x_nat[0:r, t * D + oc * P:t * D + (oc + 1) * P],
                               iden_b[0:r, 0:r]),
                           key=("pe", "A_tr", t) if oc == 1 else None)
                if t == chunk_tiles[c][-1]:
                    if S.plan is None:
                        S.ev[("pe", "A_chunk", c)] = S.ev[("pe", "A_tr", t)]
            # copies xtp -> xts, alternating DVE / ACT by tile parity
        for t in range(T):
            eng, sem = ("dve", "dve") if t % 2 == 0 else ("act", "act")
            S.wait(eng, ("pe", "A_tr", t))
            S.inst(eng, sem,
                   (lambda t=t: nc.vector.tensor_copy(
                       xts[:, t * D:(t + 1) * D], xtp[t % 2][:, :]))
                   if t % 2 == 0 else
                   (lambda t=t: nc.scalar.copy(
                       xts[:, t * D:(t + 1) * D], xtp[t % 2][:, :])),
                   key=("cp", "A_copy", t))
        # colsum -> AR
        S.wait("act", ("pe", "A_cs", T - 1))
        S.inst("act", "act",
               lambda: nc.scalar.copy(cs_row[0:1, :], cspsum[0:1, :]),
               key=("act", "A_copy"))
        S.wait("sp", ("act", "A_copy"))
        S.inst("sp", "dma_m",
               lambda: nc.sync.dma_start(out=cs_dram[:, :], in_=cs_row[0:1, :]),
               step=16, key=("dma", "cs_out"))
        S.wait("pool", ("dma", "cs_out"))
        S.inst("pool", "cc",
               lambda: nc.gpsimd.collective_compute(
                   "AllReduce", mybir.AluOpType.add,
                   replica_groups=[list(range(n_cores))],
                   ins=[cs_dram[:, :]], outs=[cs_ar[:, :]]),
               key=("cc", "ar1"))
        h_phase(S, "h1", ("cc", "ar1"), cs_col, col_idx_h1)
        # ---- phase B ----
        pass_bc(S, "B")
        # out1 -> AR
        S.wait("act", ("pe", "B_mmOut", T - 1))
        S.inst("act", "act",
               lambda: nc.scalar.copy(o1_sb[0:H, :], outacc[:, :]),
               key=("act", "o1_copy"))
        S.wait("sp", ("act", "o1_copy"))
        S.inst("sp", "dma_m",
               lambda: nc.sync.dma_start(out=o1_dram[:, :], in_=o1_sb[0:H, :]),
               step=16, key=("dma", "o1_out"))
        S.wait("pool", ("dma", "o1_out"))
        S.inst("pool", "cc",
               lambda: nc.gpsimd.collective_compute(
                   "AllReduce", mybir.AluOpType.add,
                   replica_groups=[list(range(n_cores))],
                   ins=[o1_dram[:, :]], outs=[o1_ar[:, :]]),
               key=("cc", "ar2"))
        h_phase(S, "h2", ("cc", "ar2"), cs2_col, col_idx_h2)
        # ---- phase C ----
        pass_bc(S, "C")
        # final out (stage through o1_sb)
        S.wait("act", ("pe", "C_mmOut", T - 1))
        S.inst("act", "act",
               lambda: nc.scalar.copy(o1_sb[0:H, :], outacc[:, :]),
               key=("act", "out_copy"))
        S.wait("sp", ("act", "out_copy"))
        S.inst("sp", "dma_m",
               lambda: nc.sync.dma_start(out=out_ext[:, :], in_=o1_sb[0:H, :]),
               step=16, key=("dma", "out_final"))
        S.wait("sp", ("dma", "out_final"))

    plan = Sched()
    sched(plan)
    emit = Sched(plan.ev)
    sched(emit)

    eng_map = {
        "sp": nc.sync, "pe": nc.tensor, "act": nc.scalar,
        "dve": nc.vector, "pool": nc.gpsimd,
    }

    def run_ops(eng_name):
        eng = eng_map[eng_name]
        def body(_engine):
            for op in emit.ops[eng_name]:
                if op[0] == "w":
                    _, sem, v = op
                    eng.wait_ge(sems[sem], v)
                else:
                    _, thunk, sem, step = op
                    bi = thunk()
                    bi.then_inc(sems[sem], step)
        return body

    with nc.Block() as block:
        block.sync(run_ops("sp"))
        block.gpsimd(run_ops("pool"))
        block.tensor(run_ops("pe"))
        block.scalar(run_ops("act"))
        block.vector(run_ops("dve"))

    return nc


_NC_CACHE = {}


def _get_nc(n_rows, n_cores, n_total):
    key = (n_rows, n_cores, n_total)
    if key not in _NC_CACHE:
        _NC_CACHE[key] = build_kernel(n_rows, n_cores, n_total)
    return _NC_CACHE[key]


def kernel(x, W):
    from concourse.bass_utils import run_bass_kernel_spmd

    x = np.ascontiguousarray(np.asarray(x, dtype=np.float32))
    W = np.ascontiguousarray(np.asarray(W, dtype=np.float32))
    n, d = x.shape
    assert n % N_CORES == 0 and d == D
    n_rows = n // N_CORES

    nc = _get_nc(n_rows, N_CORES, n)
    in_maps = [
        {"x": x[i * n_rows:(i + 1) * n_rows], "W": W} for i in range(N_CORES)
    ]
    res = run_bass_kernel_spmd(nc, in_maps, core_ids=list(range(N_CORES)))
    total = np.zeros((H, D), dtype=np.float64)
    for i in range(N_CORES):
        total += res.results[i]["out"].astype(np.float64)
    return total.astype(np.float32).reshape(1, H * D)


if __name__ == "__main__":
    rng = np.random.default_rng(0)
    x = rng.standard_normal((N_TOTAL, D)).astype(np.float32)
    W = (rng.standard_normal((H, D, D)) * np.sqrt(2.0 / (D + D))).astype(np.float32)
    out = kernel(x=x, W=W)
    print(out.shape, out[0, :4])
